# revision 68
# baseline (speedup 1.0000x reference)
"""Trainium2 Bass kernel for Attention3D (B=2, N=1024, C=768, H=12, HID=64).

Sharding: 8 cores = 2 batches x 4 query-slices of 256 rows.

v2: same math as the baseline (transposed attention, bias fused into scores
via PSUM accumulation), restructured for a packed PE schedule:
  - all input DMAs serialized on the SP HW queue in consumption order
    (DRAM layouts pre-arranged host-side to match SBUF, >=512B descriptors)
  - PE warmup matmuls on scratch SBUF burn the p-state ramp during the DMA
    prologue so real matmuls run at full clock
  - bank phases ordered (ic, sg, s32)-outer / jc-inner so h2 tiles are
    consumed strictly in production order (streaming through 6 buffers)
  - QKV / v-chunk / AV / transpose / proj work interleaved between bank
    phases as PE filler while DVE produces h2 and ACT drains exps
  - psum->sbuf staging split across ACT/DVE (GPSIMD cannot access PSUM on
    hardware); Pool absorbs memsets + part of the h2 stream
  - softmax normalization: one strided reciprocal (uniform 6+6 head layout),
    bulk psum->sbuf raw copy on ACT+DVE halves, then in-place bf16 scales
    that hit the DVE 4x perf mode; ic0's O-transpose is a single DMA xbar
    transpose (out[p,cc,i] = an[i,cc*128+p]) on the idle DMA device; ic1's
    tail transposes run from the idle scores psum pool so both projection
    groups interleave with the normalize chain
Host: input marshalling (transpose/rotate/scale/pack) + concat + proj_b add.
"""

import os
import sys

for _p in ("/opt/trn_rl_repo",):
    if _p not in sys.path:
        sys.path.insert(0, _p)

import numpy as np
import ml_dtypes

from contextlib import ExitStack

import concourse.bass as bass
import concourse.bacc as bacc
import concourse.mybir as mybir
import concourse.tile as tile
from concourse import bass_utils
from concourse.masks import make_identity
from bass_rust import add_dep_helper as _add_dep


def _dep(a, b):
    """a must execute after b (same-engine ordering, no semaphore)."""
    _add_dep(a.ins, b.ins, sync=False, reason="psum-accum-order")

BF16 = mybir.dt.bfloat16
F32 = mybir.dt.float32
ALU = mybir.AluOpType
ACTF = mybir.ActivationFunctionType

B, N, C, H, HID = 2, 1024, 768, 12, 64
HD = C // H  # 64
NSLICE = 4
I_LEN = N // NSLICE  # 256
P = 128

N_WARM = 45          # PE warmup matmuls (p-state ramp burn)
H2_BUFS = 6          # h2 streaming buffers
POOL_PGS = (47, 54, 55, 62, 63, 70, 71, 78, 79, 86, 87, 94, 95, 102, 103, 110, 111, 118, 119, 126, 127)        # h2 pair-groups offloaded to Pool engine

LAST_EXEC_NS = None
LAST_RESULTS = None
MM_LABELS = []  # phase label per emitted PE matmul/transpose (sim analysis)

_CACHE = {}


def avcol(h):
    return h * 65 if h < 6 else 512 + (h - 6) * 65


def _build_program():
    MM_LABELS.clear()
    nc = bacc.Bacc(
        "TRN2",
        target_bir_lowering=False,
        debug=False,
        enable_asserts=False,
        num_devices=8,
    )

    # DRAM I/O (per-core inputs; same names for all cores).
    # Weight layouts are pre-arranged host-side:
    #   kw6/qw6: [P, fc, cc*128+k]  (fc-major so per-fc slices are contiguous)
    #   vw6/pw6: [P, cc, f]         (cc-major)
    #   xT6:     [P, cc, n]         (token-rotated)
    xT6 = nc.dram_tensor("xT6", [P, 6 * N], BF16, kind="ExternalInput").ap()
    qw6 = nc.dram_tensor("qw6", [P, 6 * C], BF16, kind="ExternalInput").ap()
    kw6 = nc.dram_tensor("kw6", [P, 6 * C], BF16, kind="ExternalInput").ap()
    vw6 = nc.dram_tensor("vw6", [P, 6 * C], BF16, kind="ExternalInput").ap()
    pw6 = nc.dram_tensor("pw6", [P, 6 * C], BF16, kind="ExternalInput").ap()
    ptn2 = nc.dram_tensor("ptn2", [P, N], BF16, kind="ExternalInput").ap()
    at2 = nc.dram_tensor("at2", [P, I_LEN // 2], F32, kind="ExternalInput").ap()
    w2pk = nc.dram_tensor("w2pk", [P, 2 * H], BF16, kind="ExternalInput").ap()
    out = nc.dram_tensor("out", [I_LEN, C], BF16, kind="ExternalOutput").ap()

    xT6v = xT6.rearrange("p (c n) -> p c n", c=6)
    qw6v = qw6.rearrange("p (f k) -> p f k", f=6)
    kw6v = kw6.rearrange("p (f k) -> p f k", f=6)
    vw6v = vw6.rearrange("p (c f) -> p c f", c=6)
    pw6v = pw6.rearrange("p (c f) -> p c f", c=6)

    with tile.TileContext(nc) as tc, ExitStack() as ctx:
        consts = ctx.enter_context(tc.tile_pool(name="consts", bufs=1))
        h2p = ctx.enter_context(tc.tile_pool(name="h2p", bufs=H2_BUFS))
        expp = ctx.enter_context(tc.tile_pool(name="expp", bufs=1))
        anp = ctx.enter_context(tc.tile_pool(name="anp", bufs=1))
        outp = ctx.enter_context(tc.tile_pool(name="outp", bufs=2))
        rcp = ctx.enter_context(tc.tile_pool(name="rcp", bufs=1))
        ssp = ctx.enter_context(tc.tile_pool(name="ssp", bufs=4, space="PSUM"))
        avp = ctx.enter_context(tc.tile_pool(name="avp", bufs=1, space="PSUM"))
        mmp = ctx.enter_context(tc.tile_pool(name="mmp", bufs=2, space="PSUM"))

        # ---- persistent SBUF tiles ----
        ptn2_sb = consts.tile([P, N], BF16)
        at2_sb = consts.tile([P, I_LEN // 2], F32)
        w2pk_sb = consts.tile([P, 2 * H], BF16)
        xT_sb = consts.tile([P, 6, N], BF16)         # [p, cc, n]
        qwT_sb = consts.tile([P, 6, C], BF16)        # [p, fc, cc*128+k]
        kwT_sb = consts.tile([P, 6, C], BF16)        # [p, fc, cc*128+k]
        vwT_sb = consts.tile([P, 6, C], BF16)        # [p, cc, f]
        # pw is DMA'd into vwT's SBUF once the v projections have consumed vw
        # (last input DMA: nothing queues behind it except the out DMAs)
        pwT_sb = vwT_sb
        ident = consts.tile([P, P], BF16)
        kT_sb = consts.tile([P, 6, N], BF16)         # [p, fc, j]
        va_sb = consts.tile([P, 8, H * 65], BF16)
        qTz_sb = consts.tile([P, 2, 6, I_LEN], BF16)
        aT_sb = consts.tile([P, 6, I_LEN], BF16)
        warm_sb = consts.tile([P, 2, 64], BF16)      # memset scratch

        # ---- PE warmup: burn the p-state ramp during the DMA prologue ----
        nc.gpsimd.memset(warm_sb[:], 0.0)
        for w in range(N_WARM):
            ps = mmp.tile([P, P], F32, tag="mm")
            nc.tensor.matmul(
                ps[0:64, 0:64], warm_sb[:, 0, :], warm_sb[:, 1, :],
                start=True, stop=True,
            )
            MM_LABELS.append("warm")

        # ---- input DMAs, all on the SP HW queue in consumption order ----
        nc.sync.dma_start(at2_sb[:], at2)
        nc.sync.dma_start(ptn2_sb[:], ptn2)
        nc.sync.dma_start(kwT_sb[:, 0, :], kw6v[:, 0, :])
        for cc in range(6):
            nc.sync.dma_start(xT_sb[:, cc, 0:512], xT6v[:, cc, 0:512])
        nc.sync.dma_start(w2pk_sb[:], w2pk)
        nc.sync.dma_start(kwT_sb[:, 1:2, :], kw6v[:, 1:2, :])
        nc.sync.dma_start(kwT_sb[:, 2:3, :], kw6v[:, 2:3, :])
        nc.sync.dma_start(kwT_sb[:, 3:6, :], kw6v[:, 3:6, :])
        nc.sync.dma_start(qwT_sb[:, 0:2, :], qw6v[:, 0:2, :])
        nc.sync.dma_start(qwT_sb[:, 2:4, :], qw6v[:, 2:4, :])
        nc.sync.dma_start(qwT_sb[:, 4:6, :], qw6v[:, 4:6, :])
        nc.sync.dma_start(xT_sb[:, :, 512:1024], xT6v[:, :, 512:1024])
        nc.sync.dma_start(vwT_sb[:, 0:3, :], vw6v[:, 0:3, :])
        nc.sync.dma_start(vwT_sb[:, 3:6, :], vw6v[:, 3:6, :])

        # ---- Pool prologue: memsets + identity ----
        nc.gpsimd.memset(qTz_sb[64:128, 0, :, :], 0.0)
        nc.gpsimd.memset(qTz_sb[0:64, 1, :, :], 0.0)
        nc.gpsimd.memset(
            va_sb[:].rearrange("p t (h e) -> p t h e", h=H, e=65)[:, :, :, 64:65],
            1.0,
        )
        make_identity(nc, ident[:])

        # ---- DVE: h2 production (streamed through H2_BUFS buffers) ----
        # h2[(slot,d), j] = relu(A[i,d] - P[d,j]) per i-pair, 8 pairs/tile.
        h2t = []

        def emit_h2_tiles(t0, t1):
            for t in range(t0, t1):
                ht = h2p.tile([P, 8, N], BF16, tag="h2")
                for s in range(8):
                    pg = t * 8 + s
                    eng = nc.gpsimd if pg in POOL_PGS else nc.vector
                    eng.tensor_scalar(
                        ht[:, s, :], ptn2_sb[:], at2_sb[:, pg:pg + 1], 0.0,
                        ALU.add, ALU.max,
                    )
                h2t.append(ht)

        emit_h2_tiles(0, 13)

        # ---- PE building blocks ----
        def emit_kt_group(fc, jh):
            ps = mmp.tile([P, 512], F32, tag="mm")
            for cc in range(6):
                nc.tensor.matmul(
                    ps[:],
                    kwT_sb[:, fc, cc * P:(cc + 1) * P],
                    xT_sb[:, cc, jh * 512:(jh + 1) * 512],
                    start=(cc == 0),
                    stop=(cc == 5),
                )
            MM_LABELS.extend(["kT%d" % jh] * 6)
            if jh == 1:
                nc.vector.tensor_copy(kT_sb[:, fc, jh * 512:(jh + 1) * 512], ps[:])
            else:
                nc.scalar.copy(kT_sb[:, fc, jh * 512:(jh + 1) * 512], ps[:])

        def emit_qt_group(fc):
            ps = mmp.tile([P, I_LEN], F32, tag="mm")
            for cc in range(6):
                nc.tensor.matmul(
                    ps[:],
                    qwT_sb[:, fc, cc * P:(cc + 1) * P],
                    xT_sb[:, cc, 0:I_LEN],
                    start=(cc == 0),
                    stop=(cc == 5),
                )
            MM_LABELS.extend(["qT"] * 6)
            nc.scalar.copy(qTz_sb[0:64, 0, fc, :], ps[0:64, :])
            nc.scalar.copy(qTz_sb[64:128, 1, fc, :], ps[64:128, :])

        def emit_v_group(tci, oh):
            ps = mmp.tile([P, 384], F32, tag="mm")
            for cc in range(6):
                nc.tensor.matmul(
                    ps[:],
                    xT_sb[:, cc, tci * P:(tci + 1) * P],
                    vwT_sb[:, cc, oh * 384:(oh + 1) * 384],
                    start=(cc == 0),
                    stop=(cc == 5),
                )
            MM_LABELS.extend(["v"] * 6)
            dst = va_sb[:, tci, oh * 390:oh * 390 + 390].rearrange(
                "p (h e) -> p h e", h=6, e=65
            )[:, :, 0:64]
            if (tci + oh) % 2 == 0:
                nc.vector.tensor_copy(dst, ps[:])
            else:
                nc.scalar.copy(dst, ps[:])

        # expt staging tiles, one per ic (1 buffer: ic1 WAR-waits on AV ic0)
        expt_of = {}

        def emit_bank(ic, sg, s32, jc):
            """bias+scores into one psum bank, then exp -> expt staging."""
            expt = expt_of[ic]
            SS = ssp.tile([P, 512], F32, tag="ss", name="ss1")
            starter = None
            bias_mms = []
            for ph in range(17):
                if ph < 16:
                    pg = ic * 64 + sg * 32 + s32 * 16 + ph
                    ht = h2t[pg // 8]
                    lhsT = ht[:, pg % 8, jc * P:(jc + 1) * P]
                    wid = 24
                else:
                    # dummy block at cols 384..392 keeps the strided scores
                    # APs' psum bytes non-pending (hw pending-zero semantics)
                    lhsT = ptn2_sb[:, 0:P]
                    wid = 9
                mm = nc.tensor.matmul(
                    SS[:, 24 * ph:24 * ph + wid],
                    lhsT,
                    w2pk_sb[:, 0:wid],
                    start=(ph == 0),
                    stop=False,
                    skip_group_check=True,
                )
                if starter is None:
                    starter = mm
                else:
                    _dep(mm, starter)
                bias_mms.append(mm)
            # scores accumulate on top: col = il*12 + h
            ssb = SS[:, 0:384].rearrange(
                "p (a b h) -> p a b h", a=2, b=16, h=H
            )
            i0 = ic * 128 + sg * 64 + s32 * 32
            prev = None
            for h in range(12):
                fc = h // 2
                mm = nc.tensor.matmul(
                    ssb[:, :, :, h],
                    kT_sb[:, fc, jc * P:(jc + 1) * P],
                    qTz_sb[:, h % 2, fc, i0:i0 + 32],
                    start=False,
                    stop=(h == 11),
                    skip_group_check=True,
                )
                if prev is None:
                    for bm in bias_mms:
                        _dep(mm, bm)
                else:
                    _dep(mm, prev)
                prev = mm
            MM_LABELS.extend(["bank%d%d%d" % (ic, sg, s32)] * 29)
            # exp -> expt staging: col = h*128 + sg*64 + s32*32 + il
            dst = expt[:, jc, :].rearrange(
                "p (h g i) -> p g i h", h=H, g=4, i=32
            )[:, sg * 2 + s32]
            nc.scalar.activation(
                dst, SS[:, 0:384], ACTF.Exp, bias=0.0, scale=1.0
            )

        av_state = {}

        def emit_av_block(ic, jc):
            """12 AV matmuls for one (ic, jc); accumulates over jc in avp."""
            expt = expt_of[ic]
            avps, av_prev = av_state[ic]
            for h in range(12):
                hc = avcol(h)
                mm = nc.tensor.matmul(
                    avps[:, hc:hc + 65],
                    expt[:, jc, h * 128:(h + 1) * 128],
                    va_sb[:, jc, h * 65:h * 65 + 65],
                    start=(jc == 0 and h in (0, 6)),
                    stop=(jc == 7 and h in (5, 11)),
                    skip_group_check=True,
                )
                if av_prev is not None:
                    _dep(mm, av_prev)
                av_prev = mm
            MM_LABELS.extend(["AV%d" % ic] * 12)
            av_state[ic] = (avps, av_prev)

        def emit_recip_norm(ic):
            """reciprocal of row sums + normalize-copy psum->an (ACT/DVE)."""
            avps, _ = av_state[ic]
            rc = rcp.tile([P, H], F32, tag="rc")
            avv = avps[:].rearrange("p (b c) -> p b c", b=2, c=512)[
                :, :, 0:390].rearrange("p b (h e) -> p b h e", h=6, e=65)
            nc.vector.reciprocal(
                rc[:].rearrange("p (b h) -> p b h", b=2), avv[:, :, :, 64])
            # bulk psum->sbuf raw copy (DVE/ACT halves), then normalize
            # in-place: bf16 SBUF operands qualify for the DVE 4x perf mode
            an = anp.tile([P, C], BF16, tag="an")
            anv = an[:].rearrange("p (b h e) -> p b h e", b=2, h=6, e=64)
            nc.vector.tensor_copy(anv[:, 0], avv[:, 0, :, 0:64])
            nc.scalar.copy(anv[:, 1], avv[:, 1, :, 0:64])
            for h in range(12):
                nc.vector.tensor_scalar_mul(
                    an[:, h * 64:(h + 1) * 64],
                    an[:, h * 64:(h + 1) * 64],
                    rc[:, h:h + 1],
                )
            return an

        def emit_transposes(ic, an):
            # one DMA xbar transpose: out[p, cc, i] = an[i, cc*128+p]
            # (runs on the idle DMA device, freeing PE/ACT/DVE mid-kernel)
            nc.sync.dma_start_transpose(
                aT_sb[:, :, ic * P:(ic + 1) * P], an[:])

        def emit_proj(ic):
            for oh in range(2):
                ps = mmp.tile([P, 384], F32, tag="mm")
                for cc in range(6):
                    nc.tensor.matmul(
                        ps[:],
                        aT_sb[:, cc, ic * P:(ic + 1) * P],
                        pwT_sb[:, cc, oh * 384:(oh + 1) * 384],
                        start=(cc == 0),
                        stop=(cc == 5),
                    )
                MM_LABELS.extend(["proj%d" % ic] * 6)
                oc = outp.tile([P, 384], BF16, tag="oc")
                nc.vector.tensor_copy(oc[:], ps[:])
                nc.sync.dma_start(
                    out[ic * P:(ic + 1) * P, oh * 384:(oh + 1) * 384],
                    oc[:],
                )

        # ================= global schedule =================
        # PE: kT jh0, qT, [phase A | kT jh1 | v chunks], B, C, D+AV(ic0),
        #     E, [epilogue ic0], F, G, H+AV(ic1), epilogue ic1
        for fc in range(6):
            emit_kt_group(fc, 0)
        for fc in range(6):
            emit_qt_group(fc)

        expt_of[0] = expp.tile([P, 8, 12 * 128], BF16, tag="exp", name="expt0")

        # phase A (ic0, sg0, s32=0), jc0-3
        for jc in range(4):
            emit_bank(0, 0, 0, jc)
        # kT jh1
        for fc in range(6):
            emit_kt_group(fc, 1)
        # v chunks 0-1
        for tci in range(2):
            for oh in range(2):
                emit_v_group(tci, oh)
        # phase A jc4-7
        for jc in range(4, 8):
            emit_bank(0, 0, 0, jc)
        # v chunks 2-3
        for tci in range(2, 4):
            for oh in range(2):
                emit_v_group(tci, oh)
        # phase B (ic0, sg0, s32=1) + v 4-7 interleaved
        for jc in range(8):
            emit_bank(0, 0, 1, jc)
            if jc % 2 == 1 and jc // 2 + 4 < 8:
                tci = jc // 2 + 4
                emit_v_group(tci, 0)
                emit_v_group(tci, 1)
        # pw overwrites vwT now that every v projection has been emitted
        nc.sync.dma_start(vwT_sb[:], pw6v[:, :, :])

        # phase C (ic0, sg1, s32=0)
        for jc in range(8):
            emit_bank(0, 1, 0, jc)
        # phase D (ic0, sg1, s32=1) + AV(ic0) staggered one jc behind
        av_state[0] = (avp.tile([P, 1024], F32, tag="av", name="avps0"), None)
        for jc in range(8):
            emit_bank(0, 1, 1, jc)
            if jc >= 1:
                emit_av_block(0, jc - 1)
        emit_av_block(0, 7)

        # epilogue ic0 on DVE before the tail h2 tiles (so the late tiles'
        # buffer WARs don't hold the ic0 normalize hostage)
        an0 = emit_recip_norm(0)
        emit_h2_tiles(13, 16)

        expt_of[1] = expp.tile([P, 8, 12 * 128], BF16, tag="exp", name="expt1")

        # phase E (ic1, sg0, s32=0)
        for jc in range(8):
            emit_bank(1, 0, 0, jc)

        # phase F (ic1, sg0, s32=1)
        for jc in range(8):
            emit_bank(1, 0, 1, jc)

        emit_transposes(0, an0)

        # phase G (ic1, sg1, s32=0)
        for jc in range(8):
            emit_bank(1, 1, 0, jc)

        emit_proj(0)

        # phase H (ic1, sg1, s32=1) + AV(ic1)
        av_state[1] = (avp.tile([P, 1024], F32, tag="av", name="avps1"), None)
        for jc in range(8):
            emit_bank(1, 1, 1, jc)
            if jc >= 1:
                emit_av_block(1, jc - 1)
        emit_av_block(1, 7)

        # epilogue ic1: interleave transposes with both proj groups per cc
        an1 = emit_recip_norm(1)
        prj = [mmp.tile([P, 384], F32, tag="mm", name="prj%d" % oh)
               for oh in range(2)]
        for cc in range(6):
            pst = ssp.tile([P, P], BF16, tag="ss", name="pst")
            nc.tensor.transpose(pst[:], an1[:, cc * P:(cc + 1) * P], ident[:])
            MM_LABELS.append("tr1")
            nc.scalar.copy(aT_sb[:, cc, P:2 * P], pst[:])
            for oh in range(2):
                nc.tensor.matmul(
                    prj[oh][:],
                    aT_sb[:, cc, P:2 * P],
                    pwT_sb[:, cc, oh * 384:(oh + 1) * 384],
                    start=(cc == 0),
                    stop=(cc == 5),
                )
            MM_LABELS.extend(["proj1"] * 2)
        for oh in range(2):
            oc = outp.tile([P, 384], BF16, tag="oc")
            if oh == 0:
                nc.vector.tensor_copy(oc[:], prj[oh][:])
            else:
                nc.vector.tensor_copy(oc[:, 0:192], prj[oh][:, 0:192])
                nc.scalar.copy(oc[:, 192:384], prj[oh][:, 192:384])
            nc.sync.dma_start(
                out[P:2 * P, oh * 384:(oh + 1) * 384],
                oc[:],
            )

    nc.compile()
    return nc


def _prep_inputs(x, coords_3d, qkv_w, proj_w, mlp_w1, mlp_b1, mlp_w2):
    bf = ml_dtypes.bfloat16
    in_maps = []
    qw = (qkv_w[0:C] * (HD ** -0.5)).astype(np.float32)
    kw = qkv_w[C:2 * C]
    vw = qkv_w[2 * C:3 * C]

    def fc_major(w):
        # [p, fc, cc*128+k] from w [f_out, c_in]: elem = w.T[cc*128+p, fc*128+k]
        return np.ascontiguousarray(
            w.T.reshape(6, P, 6, P).transpose(1, 2, 0, 3).reshape(P, 6 * C)
        ).astype(bf)

    def cc_major(w):
        # [p, cc, f] from w [f_out, c_in]: elem = w.T[cc*128+p, f]
        return np.ascontiguousarray(
            w.T.reshape(6, P, C).transpose(1, 0, 2).reshape(P, 6 * C)
        ).astype(bf)

    qw6 = fc_major(qw)
    kw6 = fc_major(kw)
    vw6 = cc_major(vw)
    pw6 = cc_major(proj_w)
    # w2pk[par2*64+d, par*12+h] = (par==par2) * w2[h, d]
    w2pk = np.zeros((P, 2 * H), np.float32)
    w2pk[0:HID, 0:H] = mlp_w2.T
    w2pk[HID:2 * HID, H:2 * H] = mlp_w2.T
    w2pk = w2pk.astype(bf)

    for b in range(B):
        cb = coords_3d[b].astype(np.float32)
        mv = cb.max(axis=0) - cb.min(axis=0) + 1e-6
        cn = cb / mv
        Pm = cn @ mlp_w1.T.astype(np.float32)          # (1024, 64)
        Am = Pm + mlp_b1.astype(np.float32)            # (1024, 64)
        nPmT = -Pm.T                                   # (64, 1024)
        xT_b = np.ascontiguousarray(x[b].T).astype(np.float32)  # (768, 1024)
        for s in range(NSLICE):
            i0 = s * I_LEN
            # token rotation: column j' holds token (j' + i0) % N
            xTr = np.roll(xT_b, -i0, axis=1)
            xT6 = np.ascontiguousarray(
                xTr.reshape(6, P, N).transpose(1, 0, 2).reshape(P, 6 * N)
            ).astype(bf)
            ptn2 = np.empty((P, N), np.float32)
            ptn2[0:HID] = np.roll(nPmT, -i0, axis=1)
            ptn2[HID:2 * HID] = ptn2[0:HID]
            at2 = np.empty((P, I_LEN // 2), np.float32)
            Al = Am[i0:i0 + I_LEN]
            at2[0:HID] = Al[0::2].T
            at2[HID:2 * HID] = Al[1::2].T
            in_maps.append({
                "xT6": xT6,
                "qw6": qw6,
                "kw6": kw6,
                "vw6": vw6,
                "pw6": pw6,
                "ptn2": ptn2.astype(bf),
                "at2": at2.astype(np.float32),
                "w2pk": w2pk,
            })
    return in_maps


def kernel(x, coords_3d, qkv_w, proj_w, proj_b, mlp_w1, mlp_b1, mlp_w2, mlp_b2):
    global LAST_EXEC_NS, LAST_RESULTS
    x = np.asarray(x, np.float32)
    coords_3d = np.asarray(coords_3d, np.float32)
    qkv_w = np.asarray(qkv_w, np.float32)
    proj_w = np.asarray(proj_w, np.float32)
    proj_b = np.asarray(proj_b, np.float32)
    mlp_w1 = np.asarray(mlp_w1, np.float32)
    mlp_b1 = np.asarray(mlp_b1, np.float32)
    mlp_w2 = np.asarray(mlp_w2, np.float32)

    if "nc" not in _CACHE:
        _CACHE["nc"] = _build_program()
    nc = _CACHE["nc"]

    in_maps = _prep_inputs(x, coords_3d, qkv_w, proj_w, mlp_w1, mlp_b1, mlp_w2)
    trace = bool(int(os.environ.get("KERNEL_TRACE", "0")))
    res = bass_utils.run_bass_kernel_spmd(
        nc, in_maps, list(range(8)), trace=trace
    )
    LAST_EXEC_NS = res.exec_time_ns
    LAST_RESULTS = res
    full = np.empty((B, N, C), np.float32)
    ci = 0
    for b in range(B):
        for s in range(NSLICE):
            full[b, s * I_LEN:(s + 1) * I_LEN] = res.results[ci]["out"]
            ci += 1
    full += proj_b[None, None, :]
    return full
